# revision 2
# baseline (speedup 1.0000x reference)
"""Trainium2 Bass kernel for windowed multi-lag autocorrelation.

Reference computation (per (batch, seq) row of x[16, 128, 8320]):
  - 64 overlapping windows of length 256, stride 128
  - per-window mean removal, hanning window
  - autocorrelation at lags 0..31, scaled by 1/256
  -> out [16, 128, 1, 64, 32]

Device formulation (quadratic op -> DFT trick so the PE does the work):
  autocorr(w)[a] = (1/N) sum_f alpha_f |DFT_N(w)|^2[f] * cos(2*pi*f*a/N)
  with N = 255 (odd -> rfft bins f=0..127 fill the 128 partitions exactly).
  N < 256+32 makes the transform circular; the aliased lags 224..255 only
  touch hanning-damped window edges (~1e-4 rel err measured).
  Mean removal + hanning fold into the forward matrices.

Mixed precision (v2):
  - cos path: bf16 operands (x bf16 stream, A_cos bf16), 2 matmuls/group.
  - sin path: fp8 e4m3 (x8 stream, A_sin fp8) via ONE DoubleRow matmul
    contracting K=256 (both window chunks as the two k-tiles; the k-stride
    in the moving AP is just the 8-column chunk offset of the layout).
    DoubleRow measures ~1.4x over the two bf16 matmuls it replaces.
  - B is split per path (the inverse accumulates cos^2 and sin^2 in
    separate matmuls anyway): B_sin is least-squares re-fit against the
    QUANTIZED A_sin so the fp8 A error is partially compensated for free.
    Simulated end-to-end rel_l2 ~1.5e-2 vs the 2e-2 gate.

Per group of 8 rows (512 windows, free-dim column n = chunk*8 + row so
both window halves are stride-1 slices xv[:,0:512] / xv[:,8:520]):
  PE:    1 fp8 DR matmul (sin) + 2 bf16 matmuls (cos, PSUM-accumulated)
         + 8 transposed inverse matmuls: stationary sq[:, 128k:128k+128]
         (cos^2 with b_cos then sin^2 with b_sin, PSUM-accumulated),
         moving B [128f, 32lags] -> out [128 windows, 32 lags].
  Act:   sq_c = square(ps_cos) (PSUM -> bf16 SBUF; Act is the only engine
         that squares straight out of PSUM) + half the sblock out-copy.
  DVE:   s_sb = copy(ps_sin) bf16 + other half of the out-copy + a small
         share of the sin^2 muls.
  GpSimd:sq_s = s_sb * s_sb for most groups (it is the slow engine,
         ~1087ns per [128,512] TT, so DVE takes every 8th).
  The inverse of group g-4 issues after group g's forwards (lag-4 software
  pipelining); 4 groups' inverse outputs share one [128, 512] PSUM bank.

DMA: two input streams on separate queues (bf16 on sync, fp8 on gpsimd)
so they run in parallel; output per-sblock bf16 on the Act queue.

Sharding: pure data parallel, 2 batches per core across 8 cores.
"""
import os

# must be set before NRT initializes: recovers cores left wedged by a
# previous crashed run (NRT_EXEC_UNIT_UNRECOVERABLE otherwise)
os.environ.setdefault("NEURON_RT_RESET_CORES", "1")

import numpy as np
import ml_dtypes

import concourse.bass as bass
import concourse.tile as tile
from concourse import mybir
from concourse.bass import AP
from concourse.bass_utils import run_bass_kernel_spmd

NUM_AUTOCORR = 32
NUM_WINDOWS = 64
WIN_LEN = 256
WIN_STRIDE = 128
NFFT = 255
NF = 128  # rfft bins 0..127 (N odd)
SEQ = 128
BATCH = 16
VALUE = (NUM_WINDOWS - 1) * WIN_STRIDE + WIN_LEN  # 8320
NCHUNK = VALUE // WIN_STRIDE  # 65
N_CORES = 8
ROWS_PER_CORE = (BATCH // N_CORES) * SEQ  # 256
G = 8  # rows per group
NGROUP = ROWS_PER_CORE // G  # 32
NW = G * NUM_WINDOWS  # 512 windows per group (matmul free dim)
GW = G * NCHUNK  # 520 columns per group in the input tile
CB_COL = 2 * 128 + 2 * NUM_AUTOCORR  # bf16 const cols (Ac1|Ac2|Bcos|Bsin)
C8_COL = 256  # fp8 const cols (A_sin k-tiles side by side)
SB = 4  # groups stacked per output super-block (PSUM partition offsets)
NSB = NGROUP // SB  # 8
# progressive input DMA chunking: each dma_start costs ~565ns on the issuing
# engine, so few big issues beat many small ones; chunk 0 carries the consts
# so the PE can start after one transfer.
IN_CHUNKS = [1, 3, 8, 10, 10]
assert sum(IN_CHUNKS) == NGROUP

F32 = mybir.dt.float32
BF16 = mybir.dt.bfloat16
FP8 = mybir.dt.float8e4
NP_FP8 = ml_dtypes.float8_e4m3  # TRN float8e4 flavor (max 240)
N_WARMUP = 6  # dummy matmuls to ramp the PE clock while input DMA runs

LAST_EXEC_NS = None


def _build_mats():
    i = np.arange(WIN_LEN)
    f = np.arange(NF)
    h = np.hanning(WIN_LEN)
    ang = 2 * np.pi * np.outer(i, f) / NFFT
    C = h[:, None] * np.cos(ang)
    S = h[:, None] * np.sin(ang)
    Sb = np.zeros_like(S)
    Sb[:, 1:] = S[:, 1:]  # sin col j holds bin f=j; col 0 is a zero pad
    Ac = C - C.mean(axis=0, keepdims=True)  # fold per-window mean removal
    As = Sb - Sb.mean(axis=0, keepdims=True)
    fa = 2 * np.pi * np.outer(f, np.arange(NUM_AUTOCORR)) / NFFT
    alpha = np.full(NF, 2.0)
    alpha[0] = 1.0
    B = alpha[:, None] * np.cos(fa) / (NFFT * WIN_LEN)

    As8 = As.astype(NP_FP8)
    As8f = As8.astype(np.float32)
    # least-squares re-fit of B_sin against the quantized A_sin: choose
    # per-bin weights D so sum_f D[f] a8_f a8_f^T best matches the exact
    # quadratic form sum_f B[f] a_f a_f^T (Frobenius LS via the Gram matrix)
    Gm = (As8f.T @ As8f) ** 2
    M = (As8f.T @ As) ** 2
    Bs = np.linalg.lstsq(Gm + 1e-9 * np.eye(NF), M @ B, rcond=None)[0]
    return (
        Ac.astype(np.float32),
        As8,
        B.astype(np.float32),
        Bs.astype(np.float32),
    )


def _split_sync_waits(nc, max_waits=1):
    """walrus in this container rejects instructions with multiple sem waits
    ("Too many sync wait commands"); split extras into single-wait NoOps."""
    ctr = [0]

    def mknop(engine, waits):
        ctr[0] += 1
        nop = mybir.InstNoOp(name=f"waitsplit-{ctr[0]}", ins=[], outs=[])
        nop.engine = engine
        nop.sync_info = mybir.SyncInfo(on_wait=list(waits), on_update=[])
        return nop

    for fn in nc.m.functions:
        for blk in fn.blocks:
            out = []
            changed = False
            for inst in blk.instructions:
                si = inst.sync_info
                waits = list(si.on_wait) if si is not None and si.on_wait else []
                if len(waits) > max_waits:
                    changed = True
                    extra, keep = waits[:-max_waits], waits[-max_waits:]
                    for k in range(0, len(extra), max_waits):
                        out.append(mknop(inst.engine, extra[k : k + max_waits]))
                    inst.sync_info = mybir.SyncInfo(
                        on_wait=keep, on_update=list(si.on_update or [])
                    )
                out.append(inst)
            if changed:
                blk.instructions = out
    return nc


def _build_kernel():
    nc = bass.Bass(target_bir_lowering=False)
    # xtb[p, CB_COL + g*520 + c*8 + r] = x[row 8g+r, 128c + p] in bf16;
    # xt8 same layout in fp8 (C8_COL const prefix). Any column-range DMA
    # slice is per-partition contiguous in DRAM.
    xtb = nc.dram_tensor("xtb", [128, CB_COL + NGROUP * GW], BF16, kind="ExternalInput")
    xt8 = nc.dram_tensor("xt8", [128, C8_COL + NGROUP * GW], FP8, kind="ExternalInput")
    out = nc.dram_tensor("out", [NSB, 128, NW], BF16, kind="ExternalOutput")

    with tile.TileContext(nc) as tc:
        with (
            tc.tile_pool(name="xinb", bufs=1) as xbpool,
            tc.tile_pool(name="xin8", bufs=1) as x8pool,
            tc.tile_pool(name="sqp", bufs=5) as sqpool,
            tc.tile_pool(name="ssb", bufs=4) as spool,
            tc.tile_pool(name="outb", bufs=2) as opool,
            tc.tile_pool(name="psf", bufs=3, space="PSUM") as pspool,
            tc.tile_pool(name="pso", bufs=2, space="PSUM") as psopool,
        ):
            # input in progressively-sized chunks on two parallel queues;
            # chunk 0 of each stream carries that stream's consts
            bchunks = []  # (tile, first_group, n_groups, col_offset)
            chunks8 = []
            g0 = 0
            for ci, sz in enumerate(IN_CHUNKS):
                colsb = sz * GW + (CB_COL if ci == 0 else 0)
                cols8 = sz * GW + (C8_COL if ci == 0 else 0)
                xb_t = xbpool.tile([128, colsb], BF16, tag=f"xb{ci}")
                x8_t = x8pool.tile([128, cols8], FP8, tag=f"x8{ci}")
                lob = 0 if ci == 0 else CB_COL + g0 * GW
                lo8 = 0 if ci == 0 else C8_COL + g0 * GW
                nc.sync.dma_start(xb_t[:], xtb.ap()[:, lob : lob + colsb])
                nc.gpsimd.dma_start(x8_t[:], xt8.ap()[:, lo8 : lo8 + cols8])
                bchunks.append((xb_t, g0, sz, CB_COL if ci == 0 else 0))
                chunks8.append((x8_t, g0, sz, C8_COL if ci == 0 else 0))
                g0 += sz

            # PE p-state ramps 0.65 -> 2.4 GHz over ~3us of busy time; burn
            # the input-DMA wait on dummy matmuls over a memset tile so real
            # work starts at full clock. memset on DVE (exits the NEFF
            # preamble early; gpsimd is busy issuing the fp8 DMA descs).
            warm = xbpool.tile([128, 128 + NW], BF16, tag="warm")
            nc.vector.memset(warm[:], 0)
            ps_warm = pspool.tile([128, 2 * NW], F32, tag="ps_cs")
            for _ in range(N_WARMUP):
                nc.tensor.matmul(
                    ps_warm[:, 0:NW], warm[:, 0:128], warm[:, 128 : 128 + NW],
                    start=True, stop=True,
                )

            cb0 = bchunks[0][0]
            a_c1 = cb0[:, 0:128]
            a_c2 = cb0[:, 128:256]
            b_cos = cb0[:, 256 : 256 + NUM_AUTOCORR]
            b_sin = cb0[:, 256 + NUM_AUTOCORR : 256 + 2 * NUM_AUTOCORR]
            c80 = chunks8[0][0]
            # stationary for the sin DoubleRow matmul: [128p, 2k, 128m]
            # over the [128, 256] const block (k-tile stride 128 cols)
            as8_base = c80[:, 0:C8_COL]
            as8_st = AP(as8_base.tensor, as8_base.offset,
                        [list(as8_base.ap[0]), [128, 2], [1, 128]])

            def group_view(chunks, g):
                for t, gg0, sz, off in chunks:
                    if gg0 <= g < gg0 + sz:
                        lo = off + (g - gg0) * GW
                        return t[:, lo : lo + GW]
                raise AssertionError

            # lag-4 software pipeline: group g's inverse issues after group
            # g+4's forward, hiding the square-chain latency from the PE
            pend = []  # [(g, sq), ...]
            psout_t = None

            def flush_inverse():
                nonlocal pend, psout_t
                if not pend:
                    return
                g, sq = pend.pop(0)
                j = g % SB
                if j == 0:
                    psout_t = psopool.tile([128, NW], F32, tag="ps_out")
                for k in range(4):
                    lo = 128 * j + 32 * k
                    nc.tensor.matmul(
                        psout_t[:, lo : lo + 32],
                        sq[:, 128 * k : 128 * k + 128], b_cos,
                        start=True, stop=False,
                    )
                    nc.tensor.matmul(
                        psout_t[:, lo : lo + 32],
                        sq[:, NW + 128 * k : NW + 128 * k + 128], b_sin,
                        start=False, stop=True,
                    )
                if j == SB - 1:
                    # sblock out-copy split between Act and DVE halves so
                    # neither drain engine eats the whole 687ns PSUM read
                    o_sb = opool.tile([128, NW], BF16, tag="o_sb")
                    nc.scalar.copy(o_sb[:, 0 : NW // 2], psout_t[:, 0 : NW // 2])
                    nc.vector.tensor_copy(o_sb[:, NW // 2 : NW], psout_t[:, NW // 2 : NW])
                    # Act DGE queue: output doesn't FIFO behind bulk input
                    nc.scalar.dma_start(out.ap()[g // SB], o_sb[:])

            for g in range(NGROUP):
                xvb = group_view(bchunks, g)
                xv8 = group_view(chunks8, g)
                ps = pspool.tile([128, 2 * NW], F32, tag="ps_cs")
                # sin first so DVE's drain can start before Act's
                mov8 = AP(xv8.tensor, xv8.offset,
                          [list(xv8.ap[0]), [G, 2], [1, NW]])
                nc.tensor.matmul(
                    ps[:, NW : 2 * NW], as8_st, mov8, start=True, stop=True,
                    perf_mode=mybir.MatmulPerfMode.DoubleRow,
                )
                nc.tensor.matmul(
                    ps[:, 0:NW], a_c1, xvb[:, 0:NW], start=True, stop=False
                )
                nc.tensor.matmul(
                    ps[:, 0:NW], a_c2, xvb[:, G : G + NW], start=False, stop=True
                )

                if len(pend) >= 4:
                    flush_inverse()

                sq = sqpool.tile([128, 2 * NW], BF16, tag="sq")
                nc.scalar.square(sq[:, 0:NW], ps[:, 0:NW])
                s_sb = spool.tile([128, NW], BF16, tag="s_sb")
                nc.vector.tensor_copy(s_sb[:], ps[:, NW : 2 * NW])
                # sin^2: gpsimd is ~1087ns per 512-wide TT, over the target
                # cadence, so DVE (438ns) takes every 8th group to let
                # gpsimd catch up
                mul_eng = nc.vector if g % 8 == 7 else nc.gpsimd
                mul_eng.tensor_mul(sq[:, NW : 2 * NW], s_sb[:], s_sb[:])
                pend.append((g, sq))

            while pend:
                flush_inverse()

    _split_sync_waits(nc)
    return nc


def _install_ntff_shim():
    """The trimmed antenv lacks axon_hooks, so trace=True degrades to no
    profile. Recreate the hook: ctypes into libaxon_pjrt.so (same ABI the
    boot shim uses), exposed as a synthetic antenv.axon_hooks module."""
    import sys
    import ctypes
    import contextlib
    import types

    if "antenv.axon_hooks" in sys.modules:
        return
    so_path = "/opt/axon/libaxon_pjrt.so"
    if not os.path.exists(so_path):
        return
    lib = ctypes.CDLL(so_path)
    if not hasattr(lib, "axon_start_nrt_profile"):
        return
    lib.axon_start_nrt_profile.argtypes = [
        ctypes.POINTER(ctypes.c_int64),
        ctypes.c_size_t,
    ]
    lib.axon_start_nrt_profile.restype = ctypes.c_int64
    lib.axon_stop_nrt_profile.argtypes = [ctypes.c_char_p]
    lib.axon_stop_nrt_profile.restype = ctypes.c_int64

    @contextlib.contextmanager
    def _hook(output_dir, device_ids):
        import jax

        jax.devices()
        if device_ids:
            ids = (ctypes.c_int64 * len(device_ids))(*device_ids)
            rc = lib.axon_start_nrt_profile(ids, len(device_ids))
        else:
            rc = lib.axon_start_nrt_profile(None, 0)
        if rc != 0:
            raise RuntimeError(f"axon_start_nrt_profile rc={rc}")
        try:
            yield
        finally:
            n = lib.axon_stop_nrt_profile(str(output_dir).encode())
            print(f"ntff profile: {n} file(s) -> {output_dir}")

    mod = types.ModuleType("antenv.axon_hooks")
    mod.get_axon_ntff_profile_hook = lambda: _hook
    mod.set_axon_ntff_profile_hook = lambda h: None
    sys.modules["antenv.axon_hooks"] = mod

    # avoid network-dependent artifact uploads in the trace path
    import concourse.bass_utils as bu

    bu.upload_artifacts = lambda tmpdir: f"local://{tmpdir}"


_NC_CACHE = None


def _get_nc():
    global _NC_CACHE
    if _NC_CACHE is None:
        _NC_CACHE = _build_kernel()
    return _NC_CACHE


def kernel(x: np.ndarray) -> np.ndarray:
    global LAST_EXEC_NS
    x = np.ascontiguousarray(np.asarray(x), dtype=np.float32)
    assert x.shape == (BATCH, SEQ, VALUE)

    Ac, As8, Bc, Bs = _build_mats()
    constsb = np.zeros((128, CB_COL), np.float32)
    constsb[:, 0:128] = Ac[0:128]
    constsb[:, 128:256] = Ac[128:256]
    constsb[:, 256 : 256 + NUM_AUTOCORR] = Bc
    constsb[:, 256 + NUM_AUTOCORR : 256 + 2 * NUM_AUTOCORR] = Bs
    constsb = constsb.astype(ml_dtypes.bfloat16)
    # fp8 consts: A_sin k-tiles side by side ([p, k*128+m] = As[k*128+p, m])
    consts8 = np.zeros((128, C8_COL), NP_FP8)
    consts8[:, 0:128] = As8[0:128]
    consts8[:, 128:256] = As8[128:256]

    bpc = BATCH // N_CORES
    in_maps = []
    for c in range(N_CORES):
        xc = x[c * bpc : (c + 1) * bpc]  # [2, 128, 8320]
        # xd[p, g, c, r] = x[row 8g+r, 128c + p]
        xd = (
            xc.reshape(NGROUP, G, NCHUNK, WIN_STRIDE)  # [g, r, c, p]
            .transpose(3, 0, 2, 1)  # [p, g, c, r]
            .reshape(128, NGROUP * GW)
        )
        xtb = np.concatenate([constsb, xd.astype(ml_dtypes.bfloat16)], axis=1)
        xt8 = np.concatenate([consts8, xd.astype(NP_FP8)], axis=1)
        in_maps.append(
            {"xtb": np.ascontiguousarray(xtb), "xt8": np.ascontiguousarray(xt8)}
        )

    nc = _get_nc()
    trace = os.environ.get("AUTOCORR_TRACE", "0") == "1"
    if trace:
        _install_ntff_shim()
    try:
        res = run_bass_kernel_spmd(
            nc, in_maps, core_ids=list(range(N_CORES)), trace=trace
        )
    except Exception:
        # a stale/wedged device occasionally fails the first exec after a
        # fresh NEFF load; one retry has always recovered it
        res = run_bass_kernel_spmd(
            nc, in_maps, core_ids=list(range(N_CORES)), trace=trace
        )
    LAST_EXEC_NS = res.exec_time_ns

    outs = []
    for c in range(N_CORES):
        o = np.asarray(res.results[c]["out"]).astype(np.float32)
        # [sblock, partition p, col 128j+32k+a] where window-in-group
        # c*8+r = 128k+p, i.e. w = 16k + p//8, r = p%8, group = 4s+j
        o = o.reshape(NSB, 16, G, SB, 4, NUM_AUTOCORR)  # [s, pq, rp, j, k, a]
        o = o.transpose(0, 3, 2, 4, 1, 5)  # [s, j, rp, k, pq, a]
        outs.append(o.reshape(bpc, SEQ, NUM_WINDOWS, NUM_AUTOCORR))
    full = np.concatenate(outs, axis=0)  # [16, 128, 64, 32]
    return np.ascontiguousarray(full[:, :, None, :, :])


# revision 8
# speedup vs baseline: 1.0533x; 1.0533x over previous
"""Trainium2 Bass kernel for windowed multi-lag autocorrelation.

Reference computation (per (batch, seq) row of x[16, 128, 8320]):
  - 64 overlapping windows of length 256, stride 128
  - per-window mean removal, hanning window
  - autocorrelation at lags 0..31, scaled by 1/256
  -> out [16, 128, 1, 64, 32]

Device formulation (quadratic op -> DFT trick so the PE does the work):
  autocorr(w)[a] = (1/N) sum_f alpha_f |DFT_N(w)|^2[f] * cos(2*pi*f*a/N)
  with N = 255 (odd -> rfft bins f=0..127 fill the 128 partitions exactly).
  N < 256+32 makes the transform circular; the aliased lags 224..255 only
  touch hanning-damped window edges (~1e-4 rel err measured).
  Mean removal + hanning fold into the forward matrices.

Mixed precision (v2):
  - cos path: bf16 operands (x bf16 stream, A_cos bf16), 2 matmuls/group.
  - sin path: fp8 e4m3 (x8 stream, A_sin fp8) via ONE DoubleRow matmul
    contracting K=256 (both window chunks as the two k-tiles; the k-stride
    in the moving AP is just the 8-column chunk offset of the layout).
    DoubleRow measures ~1.4x over the two bf16 matmuls it replaces.
  - B is split per path (the inverse accumulates cos^2 and sin^2 in
    separate matmuls anyway): B_sin is least-squares re-fit against the
    QUANTIZED A_sin so the fp8 A error is partially compensated for free.
    Simulated end-to-end rel_l2 ~1.5e-2 vs the 2e-2 gate.

Per group of 8 rows (512 windows, free-dim column n = chunk*8 + row so
both window halves are stride-1 slices xv[:,0:512] / xv[:,8:520]):
  PE:    1 fp8 DR matmul (sin) + 2 bf16 matmuls (cos, PSUM-accumulated)
         + 8 transposed inverse matmuls: stationary sq[:, 128k:128k+128]
         (cos^2 with b_cos then sin^2 with b_sin, PSUM-accumulated),
         moving B [128f, 32lags] -> out [128 windows, 32 lags].
  Act:   sq_c = square(ps_cos) (PSUM -> bf16 SBUF; Act is the only engine
         that squares straight out of PSUM) + half the sblock out-copy.
  DVE:   s_sb = copy(ps_sin) bf16 + other half of the out-copy + a small
         share of the sin^2 muls.
  GpSimd:sq_s = s_sb * s_sb for most groups (it is the slow engine,
         ~1087ns per [128,512] TT, so DVE takes every 8th).
  The inverse of group g-4 issues after group g's forwards (lag-4 software
  pipelining); 4 groups' inverse outputs share one [128, 512] PSUM bank.

DMA: two input streams on separate queues (bf16 on sync, fp8 on gpsimd)
so they run in parallel; output per-sblock bf16 on the Act queue.

Sharding: pure data parallel, 2 batches per core across 8 cores.
"""
import os

# must be set before NRT initializes: recovers cores left wedged by a
# previous crashed run (NRT_EXEC_UNIT_UNRECOVERABLE otherwise)
os.environ.setdefault("NEURON_RT_RESET_CORES", "1")

import numpy as np
import ml_dtypes

import concourse.bass as bass
import concourse.tile as tile
from concourse import mybir
from concourse.bass import AP
from concourse.bass_utils import run_bass_kernel_spmd

NUM_AUTOCORR = 32
NUM_WINDOWS = 64
WIN_LEN = 256
WIN_STRIDE = 128
NFFT = 255
NF = 128  # rfft bins 0..127 (N odd)
SEQ = 128
BATCH = 16
VALUE = (NUM_WINDOWS - 1) * WIN_STRIDE + WIN_LEN  # 8320
NCHUNK = VALUE // WIN_STRIDE  # 65
N_CORES = 8
ROWS_PER_CORE = (BATCH // N_CORES) * SEQ  # 256
G = 8  # rows per group
NGROUP = ROWS_PER_CORE // G  # 32
NW = G * NUM_WINDOWS  # 512 windows per group (matmul free dim)
GW = G * NCHUNK  # 520 columns per group in the input tile
CB_COL = 2 * 128 + 2 * NUM_AUTOCORR  # bf16 const cols (Ac1|Ac2|Bcos|Bsin)
C8_COL = 256  # fp8 const cols (A_sin k-tiles side by side)
SB = 4  # groups stacked per output super-block (PSUM partition offsets)
NSB = NGROUP // SB  # 8
# progressive input DMA chunking: each dma_start costs ~565ns on the issuing
# engine, so few big issues beat many small ones; chunk 0 carries ONLY the
# consts (tiny, lands fast) so the PE can start after two transfers.
IN_CHUNKS = [0, 2, 4, 8, 9, 9]
assert sum(IN_CHUNKS) == NGROUP

F32 = mybir.dt.float32
BF16 = mybir.dt.bfloat16
FP8 = mybir.dt.float8e4
NP_FP8 = ml_dtypes.float8_e4m3  # TRN float8e4 flavor (max 240)
N_WARMUP = 6  # dummy matmuls to ramp the PE clock while input DMA runs

LAST_EXEC_NS = None


def _build_mats():
    i = np.arange(WIN_LEN)
    f = np.arange(NF)
    h = np.hanning(WIN_LEN)
    ang = 2 * np.pi * np.outer(i, f) / NFFT
    C = h[:, None] * np.cos(ang)
    S = h[:, None] * np.sin(ang)
    Sb = np.zeros_like(S)
    Sb[:, 1:] = S[:, 1:]  # sin col j holds bin f=j; col 0 is a zero pad
    Ac = C - C.mean(axis=0, keepdims=True)  # fold per-window mean removal
    As = Sb - Sb.mean(axis=0, keepdims=True)
    fa = 2 * np.pi * np.outer(f, np.arange(NUM_AUTOCORR)) / NFFT
    alpha = np.full(NF, 2.0)
    alpha[0] = 1.0
    B = alpha[:, None] * np.cos(fa) / (NFFT * WIN_LEN)

    As8 = As.astype(NP_FP8)
    As8f = As8.astype(np.float32)
    # least-squares re-fit of B_sin against the quantized A_sin: choose
    # per-bin weights D so sum_f D[f] a8_f a8_f^T best matches the exact
    # quadratic form sum_f B[f] a_f a_f^T (Frobenius LS via the Gram matrix)
    Gm = (As8f.T @ As8f) ** 2
    M = (As8f.T @ As) ** 2
    Bs = np.linalg.lstsq(Gm + 1e-9 * np.eye(NF), M @ B, rcond=None)[0]
    return (
        Ac.astype(np.float32),
        As8,
        B.astype(np.float32),
        Bs.astype(np.float32),
    )


def _split_sync_waits(nc, max_waits=1):
    """walrus in this container rejects instructions with multiple sem waits
    ("Too many sync wait commands"); split extras into single-wait NoOps."""
    ctr = [0]

    def mknop(engine, waits):
        ctr[0] += 1
        nop = mybir.InstNoOp(name=f"waitsplit-{ctr[0]}", ins=[], outs=[])
        nop.engine = engine
        nop.sync_info = mybir.SyncInfo(on_wait=list(waits), on_update=[])
        return nop

    for fn in nc.m.functions:
        for blk in fn.blocks:
            out = []
            changed = False
            for inst in blk.instructions:
                si = inst.sync_info
                waits = list(si.on_wait) if si is not None and si.on_wait else []
                if len(waits) > max_waits:
                    changed = True
                    extra, keep = waits[:-max_waits], waits[-max_waits:]
                    for k in range(0, len(extra), max_waits):
                        out.append(mknop(inst.engine, extra[k : k + max_waits]))
                    inst.sync_info = mybir.SyncInfo(
                        on_wait=keep, on_update=list(si.on_update or [])
                    )
                out.append(inst)
            if changed:
                blk.instructions = out
    return nc


def _build_kernel():
    nc = bass.Bass(target_bir_lowering=False)
    # xtb[p, CB_COL + g*520 + c*8 + r] = x[row 8g+r, 128c + p] in bf16;
    # xt8 same layout in fp8 (C8_COL const prefix). Any column-range DMA
    # slice is per-partition contiguous in DRAM.
    xtb = nc.dram_tensor("xtb", [128, CB_COL + NGROUP * GW], BF16, kind="ExternalInput")
    xt8 = nc.dram_tensor("xt8", [128, C8_COL + NGROUP * GW], FP8, kind="ExternalInput")
    out = nc.dram_tensor("out", [NSB, 128, NW], BF16, kind="ExternalOutput")

    with tile.TileContext(nc) as tc:
        with (
            tc.tile_pool(name="xinb", bufs=1) as xbpool,
            tc.tile_pool(name="xin8", bufs=1) as x8pool,
            tc.tile_pool(name="sqp", bufs=5) as sqpool,
            tc.tile_pool(name="ssb", bufs=4) as spool,
            # one SBUF tile per sblock: the output DMAs ride the sync queue
            # behind the bulk input, so the staging tiles must stay live
            # until ~2/3 through the run
            tc.tile_pool(name="outb", bufs=NSB) as opool,
            tc.tile_pool(name="psf", bufs=3, space="PSUM") as pspool,
            tc.tile_pool(name="pso", bufs=2, space="PSUM") as psopool,
        ):
            # input in progressively-sized chunks on two parallel queues;
            # chunk 0 of each stream carries that stream's consts
            bchunks = []  # (tile, first_group, n_groups, col_offset)
            chunks8 = []
            g0 = 0
            for ci, sz in enumerate(IN_CHUNKS):
                colsb = sz * GW + (CB_COL if ci == 0 else 0)
                cols8 = sz * GW + (C8_COL if ci == 0 else 0)
                xb_t = xbpool.tile([128, colsb], BF16, tag=f"xb{ci}")
                x8_t = x8pool.tile([128, cols8], FP8, tag=f"x8{ci}")
                lob = 0 if ci == 0 else CB_COL + g0 * GW
                lo8 = 0 if ci == 0 else C8_COL + g0 * GW
                nc.sync.dma_start(xb_t[:], xtb.ap()[:, lob : lob + colsb])
                nc.gpsimd.dma_start(x8_t[:], xt8.ap()[:, lo8 : lo8 + cols8])
                if sz:
                    bchunks.append((xb_t, g0, sz, CB_COL if ci == 0 else 0))
                    chunks8.append((x8_t, g0, sz, C8_COL if ci == 0 else 0))
                else:
                    consts_b_t, consts_8_t = xb_t, x8_t
                g0 += sz

            # PE p-state ramps 0.65 -> 2.4 GHz over ~3us of busy time; burn
            # the input-DMA wait on dummy matmuls over a memset tile so real
            # work starts at full clock. memset on DVE (exits the NEFF
            # preamble early; gpsimd is busy issuing the fp8 DMA descs).
            warm = xbpool.tile([128, 128 + NW], BF16, tag="warm")
            nc.vector.memset(warm[:], 0)
            ps_warm = pspool.tile([128, 2 * NW], F32, tag="ps_cs")
            for _ in range(N_WARMUP):
                nc.tensor.matmul(
                    ps_warm[:, 0:NW], warm[:, 0:128], warm[:, 128 : 128 + NW],
                    start=True, stop=True,
                )

            cb0 = consts_b_t
            a_c1 = cb0[:, 0:128]
            a_c2 = cb0[:, 128:256]
            b_cos = cb0[:, 256 : 256 + NUM_AUTOCORR]
            b_sin = cb0[:, 256 + NUM_AUTOCORR : 256 + 2 * NUM_AUTOCORR]
            c80 = consts_8_t
            # stationary for the sin DoubleRow matmul: [128p, 2k, 128m]
            # over the [128, 256] const block (k-tile stride 128 cols)
            as8_base = c80[:, 0:C8_COL]
            as8_st = AP(as8_base.tensor, as8_base.offset,
                        [list(as8_base.ap[0]), [128, 2], [1, 128]])

            def group_view(chunks, g):
                for t, gg0, sz, off in chunks:
                    if gg0 <= g < gg0 + sz:
                        lo = off + (g - gg0) * GW
                        return t[:, lo : lo + GW]
                raise AssertionError

            # lag-4 software pipeline: group g's inverse issues after group
            # g+4's forward, hiding the square-chain latency from the PE
            pend = []  # [(g, sq), ...]
            psout_t = None

            def flush_inverse():
                nonlocal pend, psout_t
                if not pend:
                    return
                g, sq = pend.pop(0)
                j = g % SB
                if j == 0:
                    psout_t = psopool.tile([128, NW], F32, tag="ps_out")
                for k in range(4):
                    lo = 128 * j + 32 * k
                    nc.tensor.matmul(
                        psout_t[:, lo : lo + 32],
                        sq[:, 128 * k : 128 * k + 128], b_cos,
                        start=True, stop=False,
                    )
                    nc.tensor.matmul(
                        psout_t[:, lo : lo + 32],
                        sq[:, NW + 128 * k : NW + 128 * k + 128], b_sin,
                        start=False, stop=True,
                    )
                if j == SB - 1:
                    # sblock out-copy split between Act and DVE halves so
                    # neither drain engine eats the whole 687ns PSUM read
                    o_sb = opool.tile([128, NW], BF16, tag="o_sb")
                    nc.scalar.copy(o_sb[:, 0 : NW // 2], psout_t[:, 0 : NW // 2])
                    nc.vector.tensor_copy(o_sb[:, NW // 2 : NW], psout_t[:, NW // 2 : NW])
                    # descriptor-gen on the idle sync engine; the transfer
                    # queues behind the remaining bulk input on that queue
                    # but still completes before the compute drains
                    nc.sync.dma_start(out.ap()[g // SB], o_sb[:])

            for g in range(NGROUP):
                xvb = group_view(bchunks, g)
                xv8 = group_view(chunks8, g)
                ps = pspool.tile([128, 2 * NW], F32, tag="ps_cs")
                # sin first so DVE's drain can start before Act's
                mov8 = AP(xv8.tensor, xv8.offset,
                          [list(xv8.ap[0]), [G, 2], [1, NW]])
                nc.tensor.matmul(
                    ps[:, NW : 2 * NW], as8_st, mov8, start=True, stop=True,
                    perf_mode=mybir.MatmulPerfMode.DoubleRow,
                )
                nc.tensor.matmul(
                    ps[:, 0:NW], a_c1, xvb[:, 0:NW], start=True, stop=False
                )
                nc.tensor.matmul(
                    ps[:, 0:NW], a_c2, xvb[:, G : G + NW], start=False, stop=True
                )

                if len(pend) >= 4:
                    flush_inverse()

                sq = sqpool.tile([128, 2 * NW], BF16, tag="sq")
                nc.scalar.square(sq[:, 0:NW], ps[:, 0:NW])
                s_sb = spool.tile([128, NW], BF16, tag="s_sb")
                nc.vector.tensor_copy(s_sb[:], ps[:, NW : 2 * NW])
                # sin^2: gpsimd is ~1051ns per 512-wide TT, over the target
                # cadence, so DVE (438ns) takes every 4th group; in the last
                # 4 groups DVE takes all of them so the drain tail is short
                mul_eng = nc.vector if (g % 4 == 3 or g >= NGROUP - 4) else nc.gpsimd
                mul_eng.tensor_mul(sq[:, NW : 2 * NW], s_sb[:], s_sb[:])
                pend.append((g, sq))

            while pend:
                flush_inverse()

    _split_sync_waits(nc)
    return nc


def _install_ntff_shim():
    """The trimmed antenv lacks axon_hooks, so trace=True degrades to no
    profile. Recreate the hook: ctypes into libaxon_pjrt.so (same ABI the
    boot shim uses), exposed as a synthetic antenv.axon_hooks module."""
    import sys
    import ctypes
    import contextlib
    import types

    if "antenv.axon_hooks" in sys.modules:
        return
    so_path = "/opt/axon/libaxon_pjrt.so"
    if not os.path.exists(so_path):
        return
    lib = ctypes.CDLL(so_path)
    if not hasattr(lib, "axon_start_nrt_profile"):
        return
    lib.axon_start_nrt_profile.argtypes = [
        ctypes.POINTER(ctypes.c_int64),
        ctypes.c_size_t,
    ]
    lib.axon_start_nrt_profile.restype = ctypes.c_int64
    lib.axon_stop_nrt_profile.argtypes = [ctypes.c_char_p]
    lib.axon_stop_nrt_profile.restype = ctypes.c_int64

    @contextlib.contextmanager
    def _hook(output_dir, device_ids):
        import jax

        jax.devices()
        if device_ids:
            ids = (ctypes.c_int64 * len(device_ids))(*device_ids)
            rc = lib.axon_start_nrt_profile(ids, len(device_ids))
        else:
            rc = lib.axon_start_nrt_profile(None, 0)
        if rc != 0:
            raise RuntimeError(f"axon_start_nrt_profile rc={rc}")
        try:
            yield
        finally:
            n = lib.axon_stop_nrt_profile(str(output_dir).encode())
            print(f"ntff profile: {n} file(s) -> {output_dir}")

    mod = types.ModuleType("antenv.axon_hooks")
    mod.get_axon_ntff_profile_hook = lambda: _hook
    mod.set_axon_ntff_profile_hook = lambda h: None
    sys.modules["antenv.axon_hooks"] = mod

    # avoid network-dependent artifact uploads in the trace path
    import concourse.bass_utils as bu

    bu.upload_artifacts = lambda tmpdir: f"local://{tmpdir}"


_NC_CACHE = None


def _get_nc():
    global _NC_CACHE
    if _NC_CACHE is None:
        _NC_CACHE = _build_kernel()
    return _NC_CACHE


def kernel(x: np.ndarray) -> np.ndarray:
    global LAST_EXEC_NS
    x = np.ascontiguousarray(np.asarray(x), dtype=np.float32)
    assert x.shape == (BATCH, SEQ, VALUE)

    Ac, As8, Bc, Bs = _build_mats()
    constsb = np.zeros((128, CB_COL), np.float32)
    constsb[:, 0:128] = Ac[0:128]
    constsb[:, 128:256] = Ac[128:256]
    constsb[:, 256 : 256 + NUM_AUTOCORR] = Bc
    constsb[:, 256 + NUM_AUTOCORR : 256 + 2 * NUM_AUTOCORR] = Bs
    constsb = constsb.astype(ml_dtypes.bfloat16)
    # fp8 consts: A_sin k-tiles side by side ([p, k*128+m] = As[k*128+p, m])
    consts8 = np.zeros((128, C8_COL), NP_FP8)
    consts8[:, 0:128] = As8[0:128]
    consts8[:, 128:256] = As8[128:256]

    bpc = BATCH // N_CORES
    in_maps = []
    for c in range(N_CORES):
        xc = x[c * bpc : (c + 1) * bpc]  # [2, 128, 8320]
        # xd[p, g, c, r] = x[row 8g+r, 128c + p]
        xd = (
            xc.reshape(NGROUP, G, NCHUNK, WIN_STRIDE)  # [g, r, c, p]
            .transpose(3, 0, 2, 1)  # [p, g, c, r]
            .reshape(128, NGROUP * GW)
        )
        xtb = np.concatenate([constsb, xd.astype(ml_dtypes.bfloat16)], axis=1)
        xt8 = np.concatenate([consts8, xd.astype(NP_FP8)], axis=1)
        in_maps.append(
            {"xtb": np.ascontiguousarray(xtb), "xt8": np.ascontiguousarray(xt8)}
        )

    nc = _get_nc()
    trace = os.environ.get("AUTOCORR_TRACE", "0") == "1"
    if trace:
        _install_ntff_shim()
    try:
        res = run_bass_kernel_spmd(
            nc, in_maps, core_ids=list(range(N_CORES)), trace=trace
        )
    except Exception:
        # a stale/wedged device occasionally fails the first exec after a
        # fresh NEFF load; one retry has always recovered it
        res = run_bass_kernel_spmd(
            nc, in_maps, core_ids=list(range(N_CORES)), trace=trace
        )
    LAST_EXEC_NS = res.exec_time_ns

    outs = []
    for c in range(N_CORES):
        o = np.asarray(res.results[c]["out"]).astype(np.float32)
        # [sblock, partition p, col 128j+32k+a] where window-in-group
        # c*8+r = 128k+p, i.e. w = 16k + p//8, r = p%8, group = 4s+j
        o = o.reshape(NSB, 16, G, SB, 4, NUM_AUTOCORR)  # [s, pq, rp, j, k, a]
        o = o.transpose(0, 3, 2, 4, 1, 5)  # [s, j, rp, k, pq, a]
        outs.append(o.reshape(bpc, SEQ, NUM_WINDOWS, NUM_AUTOCORR))
    full = np.concatenate(outs, axis=0)  # [16, 128, 64, 32]
    return np.ascontiguousarray(full[:, :, None, :, :])


# revision 14
# speedup vs baseline: 1.0574x; 1.0039x over previous
"""Trainium2 Bass kernel for windowed multi-lag autocorrelation.

Reference computation (per (batch, seq) row of x[16, 128, 8320]):
  - 64 overlapping windows of length 256, stride 128
  - per-window mean removal, hanning window
  - autocorrelation at lags 0..31, scaled by 1/256
  -> out [16, 128, 1, 64, 32]

Device formulation (quadratic op -> DFT trick so the PE does the work):
  autocorr(w)[a] = (1/N) sum_f alpha_f |DFT_N(w)|^2[f] * cos(2*pi*f*a/N)
  with N = 255 (odd -> rfft bins f=0..127 fill the 128 partitions exactly).
  N < 256+32 makes the transform circular; the aliased lags 224..255 only
  touch hanning-damped window edges (~1e-4 rel err measured).
  Mean removal + hanning fold into the forward matrices.

Mixed precision (v2):
  - cos path: bf16 operands (x bf16 stream, A_cos bf16), 2 matmuls/group.
  - sin path: fp8 e4m3 (x8 stream, A_sin fp8) via ONE DoubleRow matmul
    contracting K=256 (both window chunks as the two k-tiles; the k-stride
    in the moving AP is just the 8-column chunk offset of the layout).
    DoubleRow measures ~1.4x over the two bf16 matmuls it replaces.
  - B is split per path (the inverse accumulates cos^2 and sin^2 in
    separate matmuls anyway): B_sin is least-squares re-fit against the
    QUANTIZED A_sin so the fp8 A error is partially compensated for free.
    Simulated end-to-end rel_l2 ~1.5e-2 vs the 2e-2 gate.

Per group of 8 rows (512 windows, free-dim column n = chunk*8 + row so
both window halves are stride-1 slices xv[:,0:512] / xv[:,8:520]):
  PE:    1 fp8 DR matmul (sin) + 2 bf16 matmuls (cos, PSUM-accumulated)
         + 8 transposed inverse matmuls: stationary sq[:, 128k:128k+128]
         (cos^2 with b_cos then sin^2 with b_sin, PSUM-accumulated),
         moving B [128f, 32lags] -> out [128 windows, 32 lags].
  Act:   sq_c = square(ps_cos) (PSUM -> bf16 SBUF; Act is the only engine
         that squares straight out of PSUM) + half the sblock out-copy.
  DVE:   s_sb = copy(ps_sin) bf16 + other half of the out-copy + a small
         share of the sin^2 muls.
  GpSimd:sq_s = s_sb * s_sb for most groups (it is the slow engine,
         ~1087ns per [128,512] TT, so DVE takes every 8th).
  The inverse of group g-4 issues after group g's forwards (lag-4 software
  pipelining); 4 groups' inverse outputs share one [128, 512] PSUM bank.

DMA: two input streams on separate queues (bf16 on sync, fp8 on gpsimd)
so they run in parallel; output per-sblock bf16 on the Act queue.

Sharding: pure data parallel, 2 batches per core across 8 cores.
"""
import os

# must be set before NRT initializes: recovers cores left wedged by a
# previous crashed run (NRT_EXEC_UNIT_UNRECOVERABLE otherwise)
os.environ.setdefault("NEURON_RT_RESET_CORES", "1")

import numpy as np
import ml_dtypes

import concourse.bass as bass
import concourse.tile as tile
from concourse import mybir
from concourse.bass import AP
from concourse.bass_utils import run_bass_kernel_spmd

NUM_AUTOCORR = 32
NUM_WINDOWS = 64
WIN_LEN = 256
WIN_STRIDE = 128
NFFT = 255
NF = 128  # rfft bins 0..127 (N odd)
SEQ = 128
BATCH = 16
VALUE = (NUM_WINDOWS - 1) * WIN_STRIDE + WIN_LEN  # 8320
NCHUNK = VALUE // WIN_STRIDE  # 65
N_CORES = 8
ROWS_PER_CORE = (BATCH // N_CORES) * SEQ  # 256
G = 8  # rows per group
NGROUP = ROWS_PER_CORE // G  # 32
NW = G * NUM_WINDOWS  # 512 windows per group (matmul free dim)
GW = G * NCHUNK  # 520 columns per group in the input tile
CB_COL = 2 * 128 + 2 * NUM_AUTOCORR  # bf16 const cols (Ac1|Ac2|Bcos|Bsin)
C8_COL = 256  # fp8 const cols (A_sin k-tiles side by side)
SB = 4  # groups stacked per output super-block (PSUM partition offsets)
NSB = NGROUP // SB  # 8
# progressive input DMA chunking: each dma_start costs ~565ns on the issuing
# engine, so few big issues beat many small ones; chunk 0 carries the consts
# so the PE can start after one transfer per stream. Later chunks grow so
# the per-partition DMA runs get bigger (higher effective bandwidth).
IN_CHUNKS = [1, 3, 6, 10, 12]
assert sum(IN_CHUNKS) == NGROUP

F32 = mybir.dt.float32
BF16 = mybir.dt.bfloat16
FP8 = mybir.dt.float8e4
NP_FP8 = ml_dtypes.float8_e4m3  # TRN float8e4 flavor (max 240)
N_WARMUP = 6  # dummy matmuls to ramp the PE clock while input DMA runs

LAST_EXEC_NS = None


def _build_mats():
    i = np.arange(WIN_LEN)
    f = np.arange(NF)
    h = np.hanning(WIN_LEN)
    ang = 2 * np.pi * np.outer(i, f) / NFFT
    C = h[:, None] * np.cos(ang)
    S = h[:, None] * np.sin(ang)
    Sb = np.zeros_like(S)
    Sb[:, 1:] = S[:, 1:]  # sin col j holds bin f=j; col 0 is a zero pad
    Ac = C - C.mean(axis=0, keepdims=True)  # fold per-window mean removal
    As = Sb - Sb.mean(axis=0, keepdims=True)
    fa = 2 * np.pi * np.outer(f, np.arange(NUM_AUTOCORR)) / NFFT
    alpha = np.full(NF, 2.0)
    alpha[0] = 1.0
    B = alpha[:, None] * np.cos(fa) / (NFFT * WIN_LEN)

    As8 = As.astype(NP_FP8)
    As8f = As8.astype(np.float32)
    # least-squares re-fit of B_sin against the quantized A_sin: choose
    # per-bin weights D so sum_f D[f] a8_f a8_f^T best matches the exact
    # quadratic form sum_f B[f] a_f a_f^T (Frobenius LS via the Gram matrix)
    Gm = (As8f.T @ As8f) ** 2
    M = (As8f.T @ As) ** 2
    Bs = np.linalg.lstsq(Gm + 1e-9 * np.eye(NF), M @ B, rcond=None)[0]
    return (
        Ac.astype(np.float32),
        As8,
        B.astype(np.float32),
        Bs.astype(np.float32),
    )


def _split_sync_waits(nc, max_waits=1):
    """walrus in this container rejects instructions with multiple sem waits
    ("Too many sync wait commands"); split extras into single-wait NoOps."""
    ctr = [0]

    def mknop(engine, waits):
        ctr[0] += 1
        nop = mybir.InstNoOp(name=f"waitsplit-{ctr[0]}", ins=[], outs=[])
        nop.engine = engine
        nop.sync_info = mybir.SyncInfo(on_wait=list(waits), on_update=[])
        return nop

    for fn in nc.m.functions:
        for blk in fn.blocks:
            out = []
            changed = False
            for inst in blk.instructions:
                si = inst.sync_info
                waits = list(si.on_wait) if si is not None and si.on_wait else []
                if len(waits) > max_waits:
                    changed = True
                    extra, keep = waits[:-max_waits], waits[-max_waits:]
                    for k in range(0, len(extra), max_waits):
                        out.append(mknop(inst.engine, extra[k : k + max_waits]))
                    inst.sync_info = mybir.SyncInfo(
                        on_wait=keep, on_update=list(si.on_update or [])
                    )
                out.append(inst)
            if changed:
                blk.instructions = out
    return nc


def _build_kernel():
    nc = bass.Bass(target_bir_lowering=False)
    # xtb[p, CB_COL + g*520 + c*8 + r] = x[row 8g+r, 128c + p] in bf16;
    # xt8 same layout in fp8 (C8_COL const prefix). Any column-range DMA
    # slice is per-partition contiguous in DRAM.
    xtb = nc.dram_tensor("xtb", [128, CB_COL + NGROUP * GW], BF16, kind="ExternalInput")
    xt8 = nc.dram_tensor("xt8", [128, C8_COL + NGROUP * GW], FP8, kind="ExternalInput")
    # partition-major output: adjacent sblocks are column-adjacent, so a
    # two-sblock DMA gets 2KB per-partition runs (DRAM writes at 1KB run
    # ~40GB/s; bigger runs matter)
    out = nc.dram_tensor("out", [128, NSB * NW], BF16, kind="ExternalOutput")

    with tile.TileContext(nc) as tc:
        with (
            tc.tile_pool(name="xinb", bufs=1) as xbpool,
            tc.tile_pool(name="xin8", bufs=1) as x8pool,
            tc.tile_pool(name="sqp", bufs=5) as sqpool,
            tc.tile_pool(name="ssb", bufs=4) as spool,
            # one SBUF tile per sblock: the output DMAs ride the sync queue
            # behind the bulk input, so the staging tiles must stay live
            # until ~2/3 through the run
            tc.tile_pool(name="outb", bufs=NSB) as opool,
            tc.tile_pool(name="psf", bufs=3, space="PSUM") as pspool,
            tc.tile_pool(name="pso", bufs=2, space="PSUM") as psopool,
        ):
            # input in progressively-sized chunks on two parallel queues;
            # chunk 0 of each stream carries that stream's consts
            bchunks = []  # (tile, first_group, n_groups, col_offset)
            chunks8 = []
            g0 = 0
            for ci, sz in enumerate(IN_CHUNKS):
                colsb = sz * GW + (CB_COL if ci == 0 else 0)
                cols8 = sz * GW + (C8_COL if ci == 0 else 0)
                xb_t = xbpool.tile([128, colsb], BF16, tag=f"xb{ci}")
                x8_t = x8pool.tile([128, cols8], FP8, tag=f"x8{ci}")
                lob = 0 if ci == 0 else CB_COL + g0 * GW
                lo8 = 0 if ci == 0 else C8_COL + g0 * GW
                nc.sync.dma_start(xb_t[:], xtb.ap()[:, lob : lob + colsb])
                nc.gpsimd.dma_start(x8_t[:], xt8.ap()[:, lo8 : lo8 + cols8])
                bchunks.append((xb_t, g0, sz, CB_COL if ci == 0 else 0))
                chunks8.append((x8_t, g0, sz, C8_COL if ci == 0 else 0))
                g0 += sz
            consts_b_t, consts_8_t = bchunks[0][0], chunks8[0][0]

            # PE p-state ramps 0.65 -> 2.4 GHz over ~3us of busy time; burn
            # the input-DMA wait on dummy matmuls over a memset tile so real
            # work starts at full clock. memset on DVE (exits the NEFF
            # preamble early; gpsimd is busy issuing the fp8 DMA descs).
            warm = xbpool.tile([128, 128 + NW], BF16, tag="warm")
            nc.vector.memset(warm[:], 0)
            ps_warm = pspool.tile([128, 2 * NW], F32, tag="ps_cs")
            for _ in range(N_WARMUP):
                nc.tensor.matmul(
                    ps_warm[:, 0:NW], warm[:, 0:128], warm[:, 128 : 128 + NW],
                    start=True, stop=True,
                )

            cb0 = consts_b_t
            a_c1 = cb0[:, 0:128]
            a_c2 = cb0[:, 128:256]
            b_cos = cb0[:, 256 : 256 + NUM_AUTOCORR]
            b_sin = cb0[:, 256 + NUM_AUTOCORR : 256 + 2 * NUM_AUTOCORR]
            c80 = consts_8_t
            # stationary for the sin DoubleRow matmul: [128p, 2k, 128m]
            # over the [128, 256] const block (k-tile stride 128 cols)
            as8_base = c80[:, 0:C8_COL]
            as8_st = AP(as8_base.tensor, as8_base.offset,
                        [list(as8_base.ap[0]), [128, 2], [1, 128]])

            def group_view(chunks, g):
                for t, gg0, sz, off in chunks:
                    if gg0 <= g < gg0 + sz:
                        lo = off + (g - gg0) * GW
                        return t[:, lo : lo + GW]
                raise AssertionError

            # lag-4 software pipeline: group g's inverse issues after group
            # g+4's forward, hiding the square-chain latency from the PE
            pend = []  # [(g, sq), ...]
            psout_t = None
            o_pair = None

            def flush_inverse():
                nonlocal pend, psout_t, o_pair
                if not pend:
                    return
                g, sq = pend.pop(0)
                j = g % SB
                if j == 0:
                    psout_t = psopool.tile([128, NW], F32, tag="ps_out")
                for k in range(4):
                    lo = 128 * j + 32 * k
                    nc.tensor.matmul(
                        psout_t[:, lo : lo + 32],
                        sq[:, 128 * k : 128 * k + 128], b_cos,
                        start=True, stop=False,
                    )
                    nc.tensor.matmul(
                        psout_t[:, lo : lo + 32],
                        sq[:, NW + 128 * k : NW + 128 * k + 128], b_sin,
                        start=False, stop=True,
                    )
                sblk = g // SB
                if sblk < NSB - 1:
                    if j != SB - 1:
                        return
                    # sblock out-copy split between Act and DVE halves so
                    # neither drain engine eats the whole 687ns PSUM read;
                    # pairs share a [128, 1024] staging tile so one DMA gets
                    # 2KB per-partition runs (DRAM writes at 1KB ~40GB/s)
                    if sblk % 2 == 0:
                        o_pair = opool.tile([128, 2 * NW], BF16, tag=f"o{sblk}")
                    po = (sblk % 2) * NW
                    nc.scalar.copy(
                        o_pair[:, po : po + NW // 2], psout_t[:, 0 : NW // 2]
                    )
                    nc.vector.tensor_copy(
                        o_pair[:, po + NW // 2 : po + NW], psout_t[:, NW // 2 : NW]
                    )
                    if sblk % 2 == 1:
                        # two-sblock DMA on the idle sync engine; it queues
                        # behind the remaining bulk input on that queue but
                        # still completes well before the compute drains
                        nc.sync.dma_start(
                            out.ap()[:, (sblk - 1) * NW : (sblk + 1) * NW],
                            o_pair[:],
                        )
                    elif sblk == 6:
                        # penultimate sblock alone on the Act queue so it
                        # overlaps the final sblock's compute
                        nc.scalar.dma_start(
                            out.ap()[:, 6 * NW : 7 * NW], o_pair[:, 0:NW]
                        )
                else:
                    # final sblock: copy + DMA per inverse-group piece so
                    # the only transfer left after the last compute is 32KB
                    if j == 0:
                        o_pair = opool.tile([128, 2 * NW], BF16, tag="o7")
                    lo = 128 * j
                    if j % 2 == 0:
                        nc.scalar.copy(
                            o_pair[:, lo : lo + 128], psout_t[:, lo : lo + 128]
                        )
                    else:
                        nc.vector.tensor_copy(
                            o_pair[:, lo : lo + 128], psout_t[:, lo : lo + 128]
                        )
                    nc.scalar.dma_start(
                        out.ap()[:, 7 * NW + lo : 7 * NW + lo + 128],
                        o_pair[:, lo : lo + 128],
                    )

            for g in range(NGROUP):
                xvb = group_view(bchunks, g)
                xv8 = group_view(chunks8, g)
                ps = pspool.tile([128, 2 * NW], F32, tag="ps_cs")
                # sin first so DVE's drain can start before Act's
                mov8 = AP(xv8.tensor, xv8.offset,
                          [list(xv8.ap[0]), [G, 2], [1, NW]])
                nc.tensor.matmul(
                    ps[:, NW : 2 * NW], as8_st, mov8, start=True, stop=True,
                    perf_mode=mybir.MatmulPerfMode.DoubleRow,
                )
                nc.tensor.matmul(
                    ps[:, 0:NW], a_c1, xvb[:, 0:NW], start=True, stop=False
                )
                nc.tensor.matmul(
                    ps[:, 0:NW], a_c2, xvb[:, G : G + NW], start=False, stop=True
                )

                if len(pend) >= 4:
                    flush_inverse()

                sq = sqpool.tile([128, 2 * NW], BF16, tag="sq")
                nc.scalar.square(sq[:, 0:NW], ps[:, 0:NW])
                s_sb = spool.tile([128, NW], BF16, tag="s_sb")
                nc.vector.tensor_copy(s_sb[:], ps[:, NW : 2 * NW])
                # sin^2: gpsimd is ~1051ns per 512-wide TT, over the target
                # cadence, so DVE (438ns) takes every 4th group; in the last
                # 4 groups DVE takes all of them so the drain tail is short
                mul_eng = nc.vector if (g % 4 == 3 or g >= NGROUP - 4) else nc.gpsimd
                mul_eng.tensor_mul(sq[:, NW : 2 * NW], s_sb[:], s_sb[:])
                pend.append((g, sq))

            while pend:
                flush_inverse()

    _split_sync_waits(nc)
    return nc


def _install_ntff_shim():
    """The trimmed antenv lacks axon_hooks, so trace=True degrades to no
    profile. Recreate the hook: ctypes into libaxon_pjrt.so (same ABI the
    boot shim uses), exposed as a synthetic antenv.axon_hooks module."""
    import sys
    import ctypes
    import contextlib
    import types

    if "antenv.axon_hooks" in sys.modules:
        return
    so_path = "/opt/axon/libaxon_pjrt.so"
    if not os.path.exists(so_path):
        return
    lib = ctypes.CDLL(so_path)
    if not hasattr(lib, "axon_start_nrt_profile"):
        return
    lib.axon_start_nrt_profile.argtypes = [
        ctypes.POINTER(ctypes.c_int64),
        ctypes.c_size_t,
    ]
    lib.axon_start_nrt_profile.restype = ctypes.c_int64
    lib.axon_stop_nrt_profile.argtypes = [ctypes.c_char_p]
    lib.axon_stop_nrt_profile.restype = ctypes.c_int64

    @contextlib.contextmanager
    def _hook(output_dir, device_ids):
        import jax

        jax.devices()
        if device_ids:
            ids = (ctypes.c_int64 * len(device_ids))(*device_ids)
            rc = lib.axon_start_nrt_profile(ids, len(device_ids))
        else:
            rc = lib.axon_start_nrt_profile(None, 0)
        if rc != 0:
            raise RuntimeError(f"axon_start_nrt_profile rc={rc}")
        try:
            yield
        finally:
            n = lib.axon_stop_nrt_profile(str(output_dir).encode())
            print(f"ntff profile: {n} file(s) -> {output_dir}")

    mod = types.ModuleType("antenv.axon_hooks")
    mod.get_axon_ntff_profile_hook = lambda: _hook
    mod.set_axon_ntff_profile_hook = lambda h: None
    sys.modules["antenv.axon_hooks"] = mod

    # avoid network-dependent artifact uploads in the trace path
    import concourse.bass_utils as bu

    bu.upload_artifacts = lambda tmpdir: f"local://{tmpdir}"


_NC_CACHE = None


def _get_nc():
    global _NC_CACHE
    if _NC_CACHE is None:
        _NC_CACHE = _build_kernel()
    return _NC_CACHE


def kernel(x: np.ndarray) -> np.ndarray:
    global LAST_EXEC_NS
    x = np.ascontiguousarray(np.asarray(x), dtype=np.float32)
    assert x.shape == (BATCH, SEQ, VALUE)

    Ac, As8, Bc, Bs = _build_mats()
    constsb = np.zeros((128, CB_COL), np.float32)
    constsb[:, 0:128] = Ac[0:128]
    constsb[:, 128:256] = Ac[128:256]
    constsb[:, 256 : 256 + NUM_AUTOCORR] = Bc
    constsb[:, 256 + NUM_AUTOCORR : 256 + 2 * NUM_AUTOCORR] = Bs
    constsb = constsb.astype(ml_dtypes.bfloat16)
    # fp8 consts: A_sin k-tiles side by side ([p, k*128+m] = As[k*128+p, m])
    consts8 = np.zeros((128, C8_COL), NP_FP8)
    consts8[:, 0:128] = As8[0:128]
    consts8[:, 128:256] = As8[128:256]

    bpc = BATCH // N_CORES
    in_maps = []
    for c in range(N_CORES):
        xc = x[c * bpc : (c + 1) * bpc]  # [2, 128, 8320]
        # xd[p, g, c, r] = x[row 8g+r, 128c + p]
        xd = (
            xc.reshape(NGROUP, G, NCHUNK, WIN_STRIDE)  # [g, r, c, p]
            .transpose(3, 0, 2, 1)  # [p, g, c, r]
            .reshape(128, NGROUP * GW)
        )
        xtb = np.concatenate([constsb, xd.astype(ml_dtypes.bfloat16)], axis=1)
        xt8 = np.concatenate([consts8, xd.astype(NP_FP8)], axis=1)
        in_maps.append(
            {"xtb": np.ascontiguousarray(xtb), "xt8": np.ascontiguousarray(xt8)}
        )

    nc = _get_nc()
    trace = os.environ.get("AUTOCORR_TRACE", "0") == "1"
    if trace:
        _install_ntff_shim()
    try:
        res = run_bass_kernel_spmd(
            nc, in_maps, core_ids=list(range(N_CORES)), trace=trace
        )
    except Exception:
        # a stale/wedged device occasionally fails the first exec after a
        # fresh NEFF load; one retry has always recovered it
        res = run_bass_kernel_spmd(
            nc, in_maps, core_ids=list(range(N_CORES)), trace=trace
        )
    LAST_EXEC_NS = res.exec_time_ns

    outs = []
    for c in range(N_CORES):
        o = np.asarray(res.results[c]["out"]).astype(np.float32)
        # out[p, s*NW + 128j+32k+a] where window-in-group c*8+r = 128k+p,
        # i.e. w = 16k + p//8, r = p%8, group = 4s+j
        o = o.reshape(16, G, NSB, SB, 4, NUM_AUTOCORR)  # [pq, rp, s, j, k, a]
        o = o.transpose(2, 3, 1, 4, 0, 5)  # [s, j, rp, k, pq, a]
        outs.append(o.reshape(bpc, SEQ, NUM_WINDOWS, NUM_AUTOCORR))
    full = np.concatenate(outs, axis=0)  # [16, 128, 64, 32]
    return np.ascontiguousarray(full[:, :, None, :, :])


# revision 17
# speedup vs baseline: 1.0595x; 1.0020x over previous
"""Trainium2 Bass kernel for windowed multi-lag autocorrelation.

Reference computation (per (batch, seq) row of x[16, 128, 8320]):
  - 64 overlapping windows of length 256, stride 128
  - per-window mean removal, hanning window
  - autocorrelation at lags 0..31, scaled by 1/256
  -> out [16, 128, 1, 64, 32]

Device formulation (quadratic op -> DFT trick so the PE does the work):
  autocorr(w)[a] = (1/N) sum_f alpha_f |DFT_N(w)|^2[f] * cos(2*pi*f*a/N)
  with N = 255 (odd -> rfft bins f=0..127 fill the 128 partitions exactly).
  N < 256+32 makes the transform circular; the aliased lags 224..255 only
  touch hanning-damped window edges (~1e-4 rel err measured).
  Mean removal + hanning fold into the forward matrices.

Mixed precision (v2):
  - cos path: bf16 operands (x bf16 stream, A_cos bf16), 2 matmuls/group.
  - sin path: fp8 e4m3 (x8 stream, A_sin fp8) via ONE DoubleRow matmul
    contracting K=256 (both window chunks as the two k-tiles; the k-stride
    in the moving AP is just the 8-column chunk offset of the layout).
    DoubleRow measures ~1.4x over the two bf16 matmuls it replaces.
  - B is split per path (the inverse accumulates cos^2 and sin^2 in
    separate matmuls anyway): B_sin is least-squares re-fit against the
    QUANTIZED A_sin so the fp8 A error is partially compensated for free.
    Simulated end-to-end rel_l2 ~1.5e-2 vs the 2e-2 gate.

Per group of 8 rows (512 windows, free-dim column n = chunk*8 + row so
both window halves are stride-1 slices xv[:,0:512] / xv[:,8:520]):
  PE:    1 fp8 DR matmul (sin) + 2 bf16 matmuls (cos, PSUM-accumulated)
         + 8 transposed inverse matmuls: stationary sq[:, 128k:128k+128]
         (cos^2 with b_cos then sin^2 with b_sin, PSUM-accumulated),
         moving B [128f, 32lags] -> out [128 windows, 32 lags].
  Act:   sq_c = square(ps_cos) (PSUM -> bf16 SBUF; Act is the only engine
         that squares straight out of PSUM) + half the sblock out-copy.
  DVE:   s_sb = copy(ps_sin) bf16 + other half of the out-copy + a small
         share of the sin^2 muls.
  GpSimd:sq_s = s_sb * s_sb for most groups (it is the slow engine,
         ~1087ns per [128,512] TT, so DVE takes every 8th).
  The inverse of group g-4 issues after group g's forwards (lag-4 software
  pipelining); 4 groups' inverse outputs share one [128, 512] PSUM bank.

DMA: two input streams on separate queues (bf16 on sync, fp8 on gpsimd)
so they run in parallel; output per-sblock bf16 on the Act queue.

Sharding: pure data parallel, 2 batches per core across 8 cores.
"""
import os

# must be set before NRT initializes: recovers cores left wedged by a
# previous crashed run (NRT_EXEC_UNIT_UNRECOVERABLE otherwise)
os.environ.setdefault("NEURON_RT_RESET_CORES", "1")

import numpy as np
import ml_dtypes

import concourse.bass as bass
import concourse.tile as tile
from concourse import mybir
from concourse.bass import AP
from concourse.bass_utils import run_bass_kernel_spmd

NUM_AUTOCORR = 32
NUM_WINDOWS = 64
WIN_LEN = 256
WIN_STRIDE = 128
NFFT = 255
NF = 128  # rfft bins 0..127 (N odd)
SEQ = 128
BATCH = 16
VALUE = (NUM_WINDOWS - 1) * WIN_STRIDE + WIN_LEN  # 8320
NCHUNK = VALUE // WIN_STRIDE  # 65
N_CORES = 8
ROWS_PER_CORE = (BATCH // N_CORES) * SEQ  # 256
G = 8  # rows per group
NGROUP = ROWS_PER_CORE // G  # 32
NW = G * NUM_WINDOWS  # 512 windows per group (matmul free dim)
GW = G * NCHUNK  # 520 columns per group in the input tile
CB_COL = 2 * 128 + 2 * NUM_AUTOCORR  # bf16 const cols (Ac1|Ac2|Bcos|Bsin)
C8_COL = 256  # fp8 const cols (A_sin k-tiles side by side)
SB = 4  # groups stacked per output super-block (PSUM partition offsets)
NSB = NGROUP // SB  # 8
# progressive input DMA chunking: each dma_start costs ~565ns on the issuing
# engine, so few big issues beat many small ones; chunk 0 carries the consts
# so the PE can start after one transfer per stream. Later chunks grow so
# the per-partition DMA runs get bigger (higher effective bandwidth).
IN_CHUNKS = [1, 3, 6, 10, 12]
assert sum(IN_CHUNKS) == NGROUP

F32 = mybir.dt.float32
BF16 = mybir.dt.bfloat16
FP8 = mybir.dt.float8e4
NP_FP8 = ml_dtypes.float8_e4m3  # TRN float8e4 flavor (max 240)
N_WARMUP = 6  # dummy matmuls to ramp the PE clock while input DMA runs

LAST_EXEC_NS = None


def _build_mats():
    i = np.arange(WIN_LEN)
    f = np.arange(NF)
    h = np.hanning(WIN_LEN)
    ang = 2 * np.pi * np.outer(i, f) / NFFT
    C = h[:, None] * np.cos(ang)
    S = h[:, None] * np.sin(ang)
    Sb = np.zeros_like(S)
    Sb[:, 1:] = S[:, 1:]  # sin col j holds bin f=j; col 0 is a zero pad
    Ac = C - C.mean(axis=0, keepdims=True)  # fold per-window mean removal
    As = Sb - Sb.mean(axis=0, keepdims=True)
    fa = 2 * np.pi * np.outer(f, np.arange(NUM_AUTOCORR)) / NFFT
    alpha = np.full(NF, 2.0)
    alpha[0] = 1.0
    B = alpha[:, None] * np.cos(fa) / (NFFT * WIN_LEN)

    As8 = As.astype(NP_FP8)
    As8f = As8.astype(np.float32)
    # least-squares re-fit of B_sin against the quantized A_sin: choose
    # per-bin weights D so sum_f D[f] a8_f a8_f^T best matches the exact
    # quadratic form sum_f B[f] a_f a_f^T (Frobenius LS via the Gram matrix)
    Gm = (As8f.T @ As8f) ** 2
    M = (As8f.T @ As) ** 2
    Bs = np.linalg.lstsq(Gm + 1e-9 * np.eye(NF), M @ B, rcond=None)[0]
    return (
        Ac.astype(np.float32),
        As8,
        B.astype(np.float32),
        Bs.astype(np.float32),
    )


def _split_sync_waits(nc, max_waits=1):
    """walrus in this container rejects instructions with multiple sem waits
    ("Too many sync wait commands"); split extras into single-wait NoOps."""
    ctr = [0]

    def mknop(engine, waits):
        ctr[0] += 1
        nop = mybir.InstNoOp(name=f"waitsplit-{ctr[0]}", ins=[], outs=[])
        nop.engine = engine
        nop.sync_info = mybir.SyncInfo(on_wait=list(waits), on_update=[])
        return nop

    for fn in nc.m.functions:
        for blk in fn.blocks:
            out = []
            changed = False
            for inst in blk.instructions:
                si = inst.sync_info
                waits = list(si.on_wait) if si is not None and si.on_wait else []
                if len(waits) > max_waits:
                    changed = True
                    extra, keep = waits[:-max_waits], waits[-max_waits:]
                    for k in range(0, len(extra), max_waits):
                        out.append(mknop(inst.engine, extra[k : k + max_waits]))
                    inst.sync_info = mybir.SyncInfo(
                        on_wait=keep, on_update=list(si.on_update or [])
                    )
                out.append(inst)
            if changed:
                blk.instructions = out
    return nc


def _build_kernel():
    nc = bass.Bass(target_bir_lowering=False)
    # xtb[p, CB_COL + g*520 + c*8 + r] = x[row 8g+r, 128c + p] in bf16;
    # xt8 same layout in fp8 (C8_COL const prefix). Any column-range DMA
    # slice is per-partition contiguous in DRAM.
    xtb = nc.dram_tensor("xtb", [128, CB_COL + NGROUP * GW], BF16, kind="ExternalInput")
    xt8 = nc.dram_tensor("xt8", [128, C8_COL + NGROUP * GW], FP8, kind="ExternalInput")
    # partition-major output: adjacent sblocks are column-adjacent, so a
    # two-sblock DMA gets 2KB per-partition runs (DRAM writes at 1KB run
    # ~40GB/s; bigger runs matter)
    out = nc.dram_tensor("out", [128, NSB * NW], BF16, kind="ExternalOutput")

    with tile.TileContext(nc) as tc:
        with (
            tc.tile_pool(name="xinb", bufs=1) as xbpool,
            tc.tile_pool(name="xin8", bufs=1) as x8pool,
            tc.tile_pool(name="sqp", bufs=5) as sqpool,
            tc.tile_pool(name="ssb", bufs=4) as spool,
            # one SBUF tile per sblock: the output DMAs ride the sync queue
            # behind the bulk input, so the staging tiles must stay live
            # until ~2/3 through the run
            tc.tile_pool(name="outb", bufs=NSB) as opool,
            tc.tile_pool(name="psf", bufs=3, space="PSUM") as pspool,
            tc.tile_pool(name="pso", bufs=2, space="PSUM") as psopool,
        ):
            # input in progressively-sized chunks on two parallel queues;
            # chunk 0 of each stream carries that stream's consts
            # ALL input on the sync queue, fp8/bf16 chunks interleaved in
            # consumption order: one queue avoids the head-of-run bandwidth
            # contention between two streams, and delivery order then matches
            # the group order exactly. The gpsimd queue is freed for output.
            bchunks = []  # (tile, first_group, n_groups, col_offset)
            chunks8 = []
            g0 = 0
            for ci, sz in enumerate(IN_CHUNKS):
                colsb = sz * GW + (CB_COL if ci == 0 else 0)
                cols8 = sz * GW + (C8_COL if ci == 0 else 0)
                xb_t = xbpool.tile([128, colsb], BF16, tag=f"xb{ci}")
                x8_t = x8pool.tile([128, cols8], FP8, tag=f"x8{ci}")
                lob = 0 if ci == 0 else CB_COL + g0 * GW
                lo8 = 0 if ci == 0 else C8_COL + g0 * GW
                nc.sync.dma_start(x8_t[:], xt8.ap()[:, lo8 : lo8 + cols8])
                nc.sync.dma_start(xb_t[:], xtb.ap()[:, lob : lob + colsb])
                bchunks.append((xb_t, g0, sz, CB_COL if ci == 0 else 0))
                chunks8.append((x8_t, g0, sz, C8_COL if ci == 0 else 0))
                g0 += sz
            consts_b_t, consts_8_t = bchunks[0][0], chunks8[0][0]

            # PE p-state ramps 0.65 -> 2.4 GHz over ~3us of busy time; burn
            # the input-DMA wait on dummy matmuls over a memset tile so real
            # work starts at full clock. memset on DVE (exits the NEFF
            # preamble early; gpsimd is busy issuing the fp8 DMA descs).
            warm = xbpool.tile([128, 128 + NW], BF16, tag="warm")
            nc.vector.memset(warm[:], 0)
            ps_warm = pspool.tile([128, 2 * NW], F32, tag="ps_cs")
            for _ in range(N_WARMUP):
                nc.tensor.matmul(
                    ps_warm[:, 0:NW], warm[:, 0:128], warm[:, 128 : 128 + NW],
                    start=True, stop=True,
                )

            cb0 = consts_b_t
            a_c1 = cb0[:, 0:128]
            a_c2 = cb0[:, 128:256]
            b_cos = cb0[:, 256 : 256 + NUM_AUTOCORR]
            b_sin = cb0[:, 256 + NUM_AUTOCORR : 256 + 2 * NUM_AUTOCORR]
            c80 = consts_8_t
            # stationary for the sin DoubleRow matmul: [128p, 2k, 128m]
            # over the [128, 256] const block (k-tile stride 128 cols)
            as8_base = c80[:, 0:C8_COL]
            as8_st = AP(as8_base.tensor, as8_base.offset,
                        [list(as8_base.ap[0]), [128, 2], [1, 128]])

            def group_view(chunks, g):
                for t, gg0, sz, off in chunks:
                    if gg0 <= g < gg0 + sz:
                        lo = off + (g - gg0) * GW
                        return t[:, lo : lo + GW]
                raise AssertionError

            # lag-4 software pipeline: group g's inverse issues after group
            # g+4's forward, hiding the square-chain latency from the PE
            pend = []  # [(g, sq), ...]
            psout_t = None
            o_pair = None

            def flush_inverse():
                nonlocal pend, psout_t, o_pair
                if not pend:
                    return
                g, sq = pend.pop(0)
                j = g % SB
                if j == 0:
                    psout_t = psopool.tile([128, NW], F32, tag="ps_out")
                for k in range(4):
                    lo = 128 * j + 32 * k
                    nc.tensor.matmul(
                        psout_t[:, lo : lo + 32],
                        sq[:, 128 * k : 128 * k + 128], b_cos,
                        start=True, stop=False,
                    )
                    nc.tensor.matmul(
                        psout_t[:, lo : lo + 32],
                        sq[:, NW + 128 * k : NW + 128 * k + 128], b_sin,
                        start=False, stop=True,
                    )
                sblk = g // SB
                if sblk < NSB - 1:
                    if j != SB - 1:
                        return
                    # sblock out-copy split between Act and DVE halves so
                    # neither drain engine eats the whole 687ns PSUM read;
                    # pairs share a [128, 1024] staging tile so one DMA gets
                    # 2KB per-partition runs (DRAM writes at 1KB ~40GB/s)
                    if sblk % 2 == 0:
                        o_pair = opool.tile([128, 2 * NW], BF16, tag=f"o{sblk}")
                    po = (sblk % 2) * NW
                    nc.scalar.copy(
                        o_pair[:, po : po + NW // 2], psout_t[:, 0 : NW // 2]
                    )
                    nc.vector.tensor_copy(
                        o_pair[:, po + NW // 2 : po + NW], psout_t[:, NW // 2 : NW]
                    )
                    if sblk % 2 == 1:
                        # two-sblock DMA (2KB runs) on the gpsimd queue,
                        # which carries no input in this layout
                        nc.gpsimd.dma_start(
                            out.ap()[:, (sblk - 1) * NW : (sblk + 1) * NW],
                            o_pair[:],
                        )
                    elif sblk == 6:
                        # penultimate sblock alone on the Act queue so it
                        # overlaps the final sblock's compute
                        nc.scalar.dma_start(
                            out.ap()[:, 6 * NW : 7 * NW], o_pair[:, 0:NW]
                        )
                else:
                    # final sblock: copy + DMA per inverse-group piece so
                    # the only transfer left after the last compute is 32KB
                    if j == 0:
                        o_pair = opool.tile([128, 2 * NW], BF16, tag="o7")
                    lo = 128 * j
                    if j % 2 == 0:
                        nc.scalar.copy(
                            o_pair[:, lo : lo + 128], psout_t[:, lo : lo + 128]
                        )
                    else:
                        nc.vector.tensor_copy(
                            o_pair[:, lo : lo + 128], psout_t[:, lo : lo + 128]
                        )
                    nc.scalar.dma_start(
                        out.ap()[:, 7 * NW + lo : 7 * NW + lo + 128],
                        o_pair[:, lo : lo + 128],
                    )

            for g in range(NGROUP):
                xvb = group_view(bchunks, g)
                xv8 = group_view(chunks8, g)
                ps = pspool.tile([128, 2 * NW], F32, tag="ps_cs")
                # sin first so DVE's drain can start before Act's
                mov8 = AP(xv8.tensor, xv8.offset,
                          [list(xv8.ap[0]), [G, 2], [1, NW]])
                nc.tensor.matmul(
                    ps[:, NW : 2 * NW], as8_st, mov8, start=True, stop=True,
                    perf_mode=mybir.MatmulPerfMode.DoubleRow,
                )
                nc.tensor.matmul(
                    ps[:, 0:NW], a_c1, xvb[:, 0:NW], start=True, stop=False
                )
                nc.tensor.matmul(
                    ps[:, 0:NW], a_c2, xvb[:, G : G + NW], start=False, stop=True
                )

                if len(pend) >= 4:
                    flush_inverse()

                sq = sqpool.tile([128, 2 * NW], BF16, tag="sq")
                nc.scalar.square(sq[:, 0:NW], ps[:, 0:NW])
                s_sb = spool.tile([128, NW], BF16, tag="s_sb")
                nc.vector.tensor_copy(s_sb[:], ps[:, NW : 2 * NW])
                # sin^2: gpsimd is ~1051ns per 512-wide TT, over the target
                # cadence, so DVE (438ns) takes every 4th group; in the last
                # 4 groups alternate DVE/gpsimd so the drain tail pipelines
                if g >= NGROUP - 4:
                    mul_eng = nc.vector if g % 2 == 0 else nc.gpsimd
                else:
                    mul_eng = nc.vector if g % 4 == 3 else nc.gpsimd
                mul_eng.tensor_mul(sq[:, NW : 2 * NW], s_sb[:], s_sb[:])
                pend.append((g, sq))

            while pend:
                flush_inverse()

    _split_sync_waits(nc)
    return nc


def _install_ntff_shim():
    """The trimmed antenv lacks axon_hooks, so trace=True degrades to no
    profile. Recreate the hook: ctypes into libaxon_pjrt.so (same ABI the
    boot shim uses), exposed as a synthetic antenv.axon_hooks module."""
    import sys
    import ctypes
    import contextlib
    import types

    if "antenv.axon_hooks" in sys.modules:
        return
    so_path = "/opt/axon/libaxon_pjrt.so"
    if not os.path.exists(so_path):
        return
    lib = ctypes.CDLL(so_path)
    if not hasattr(lib, "axon_start_nrt_profile"):
        return
    lib.axon_start_nrt_profile.argtypes = [
        ctypes.POINTER(ctypes.c_int64),
        ctypes.c_size_t,
    ]
    lib.axon_start_nrt_profile.restype = ctypes.c_int64
    lib.axon_stop_nrt_profile.argtypes = [ctypes.c_char_p]
    lib.axon_stop_nrt_profile.restype = ctypes.c_int64

    @contextlib.contextmanager
    def _hook(output_dir, device_ids):
        import jax

        jax.devices()
        if device_ids:
            ids = (ctypes.c_int64 * len(device_ids))(*device_ids)
            rc = lib.axon_start_nrt_profile(ids, len(device_ids))
        else:
            rc = lib.axon_start_nrt_profile(None, 0)
        if rc != 0:
            raise RuntimeError(f"axon_start_nrt_profile rc={rc}")
        try:
            yield
        finally:
            n = lib.axon_stop_nrt_profile(str(output_dir).encode())
            print(f"ntff profile: {n} file(s) -> {output_dir}")

    mod = types.ModuleType("antenv.axon_hooks")
    mod.get_axon_ntff_profile_hook = lambda: _hook
    mod.set_axon_ntff_profile_hook = lambda h: None
    sys.modules["antenv.axon_hooks"] = mod

    # avoid network-dependent artifact uploads in the trace path
    import concourse.bass_utils as bu

    bu.upload_artifacts = lambda tmpdir: f"local://{tmpdir}"


_NC_CACHE = None


def _get_nc():
    global _NC_CACHE
    if _NC_CACHE is None:
        _NC_CACHE = _build_kernel()
    return _NC_CACHE


def kernel(x: np.ndarray) -> np.ndarray:
    global LAST_EXEC_NS
    x = np.ascontiguousarray(np.asarray(x), dtype=np.float32)
    assert x.shape == (BATCH, SEQ, VALUE)

    Ac, As8, Bc, Bs = _build_mats()
    constsb = np.zeros((128, CB_COL), np.float32)
    constsb[:, 0:128] = Ac[0:128]
    constsb[:, 128:256] = Ac[128:256]
    constsb[:, 256 : 256 + NUM_AUTOCORR] = Bc
    constsb[:, 256 + NUM_AUTOCORR : 256 + 2 * NUM_AUTOCORR] = Bs
    constsb = constsb.astype(ml_dtypes.bfloat16)
    # fp8 consts: A_sin k-tiles side by side ([p, k*128+m] = As[k*128+p, m])
    consts8 = np.zeros((128, C8_COL), NP_FP8)
    consts8[:, 0:128] = As8[0:128]
    consts8[:, 128:256] = As8[128:256]

    bpc = BATCH // N_CORES
    in_maps = []
    for c in range(N_CORES):
        xc = x[c * bpc : (c + 1) * bpc]  # [2, 128, 8320]
        # xd[p, g, c, r] = x[row 8g+r, 128c + p]
        xd = (
            xc.reshape(NGROUP, G, NCHUNK, WIN_STRIDE)  # [g, r, c, p]
            .transpose(3, 0, 2, 1)  # [p, g, c, r]
            .reshape(128, NGROUP * GW)
        )
        xtb = np.concatenate([constsb, xd.astype(ml_dtypes.bfloat16)], axis=1)
        xt8 = np.concatenate([consts8, xd.astype(NP_FP8)], axis=1)
        in_maps.append(
            {"xtb": np.ascontiguousarray(xtb), "xt8": np.ascontiguousarray(xt8)}
        )

    nc = _get_nc()
    trace = os.environ.get("AUTOCORR_TRACE", "0") == "1"
    if trace:
        _install_ntff_shim()
    try:
        res = run_bass_kernel_spmd(
            nc, in_maps, core_ids=list(range(N_CORES)), trace=trace
        )
    except Exception:
        # a stale/wedged device occasionally fails the first exec after a
        # fresh NEFF load; one retry has always recovered it
        res = run_bass_kernel_spmd(
            nc, in_maps, core_ids=list(range(N_CORES)), trace=trace
        )
    LAST_EXEC_NS = res.exec_time_ns

    outs = []
    for c in range(N_CORES):
        o = np.asarray(res.results[c]["out"]).astype(np.float32)
        # out[p, s*NW + 128j+32k+a] where window-in-group c*8+r = 128k+p,
        # i.e. w = 16k + p//8, r = p%8, group = 4s+j
        o = o.reshape(16, G, NSB, SB, 4, NUM_AUTOCORR)  # [pq, rp, s, j, k, a]
        o = o.transpose(2, 3, 1, 4, 0, 5)  # [s, j, rp, k, pq, a]
        outs.append(o.reshape(bpc, SEQ, NUM_WINDOWS, NUM_AUTOCORR))
    full = np.concatenate(outs, axis=0)  # [16, 128, 64, 32]
    return np.ascontiguousarray(full[:, :, None, :, :])
